# revision 41
# baseline (speedup 1.0000x reference)
"""Trainium2 Bass kernel for nn_ConditionalFeedForward (MoE top-2 routing).

Strategy: expert-parallel across the 8 NeuronCores — core e owns expert e's
weights. Host (numpy) gathers each expert's routed tokens (multi-hot
routing_map), pads to a common capacity CAP, and pre-quantizes operands to a
two-level fp8-e4m3 representation (hi + lo residual, SAME power-of-2 scale so
all product terms share one PSUM scale). Each core computes, for its expert:

    hT = silu(w1 @ xT) * (w3 @ xT)          # [FFN, CAP], SBUF-resident fp8
    yT = w2 @ hT                            # [DIM, CAP]

Every GEMM runs on the PE in fp8 DoubleRow mode (2 k-chunks per instruction,
0.5 cycles/row = 4x the f32r rate) with 3 compensation terms sharing one PSUM
accumulation group:

    W @ X ~= W_hi@X_hi + W_lo@X_hi + W_hi@X_lo        (drop W_lo@X_lo ~ 1e-3)

The cross terms pair (W_lo_k, X_hi_k) and (W_hi_k, X_lo_k) as the two planes
of a single DoubleRow instruction, so each GEMM costs 3 fp8 K-passes = 0.75x
the one-pass f32r time. End-to-end rel err ~1e-3 (vs 2e-2 budget).

The SwiGLU intermediate h is re-quantized to two-level fp8 on-device
(scalar: silu, vector: scaled-mul + residual-sub, gpsimd: hi cast) and stays
SBUF-resident — no DRAM staging between the two phases. Host scatter-adds
gate-weighted outputs back to the full [N_TOKENS, DIM] result.
"""

import os
import numpy as np
import ml_dtypes

import concourse.bacc as bacc
import concourse.mybir as mybir
import concourse.tile as tile
from concourse.bass_utils import run_bass_kernel_spmd

# Problem constants (hardcoded per harness contract)
NUM_EXPERTS = 8
DIM = 2048
FFN = 5632
N_CORES = 8
KD = DIM // 128    # 16 contraction chunks for GEMM1/3; output chunks for GEMM2
KF = FFN // 128    # 44 ffn chunks

F32 = mybir.dt.float32
FP8 = mybir.dt.float8e4
NP_FP8 = ml_dtypes.float8_e4m3
DR = mybir.MatmulPerfMode.DoubleRow

# Power-of-2 quantization scales. fp8 relative precision is scale-free; the
# scale only positions the distribution inside e4m3's normal range
# (2^-6 .. 240). hi and lo share the scale so all matmul terms accumulate in
# one PSUM group.
S_X = 16.0       # x ~ N(0,1): max|x|*16 ~ 90
S_W = 1024.0     # w ~ 0.02*N(0,1): max|w|*1024 ~ 115
S_H = 4.0        # h = silu(g1)*g3, |h| <~ 20: max|h|*4 ~ 80
INV_SXW = 1.0 / (S_X * S_W)          # PSUM -> g dequant (2^-14)
G3_TO_HT = S_H / (S_X * S_W)         # PSUM g3 -> g3*S_H   (2^-12)
INV_SWH = 1.0 / (S_W * S_H)          # PSUM -> y dequant   (2^-12)

# Compiled program cache keyed by CAP
_PROGRAMS = {}

# Filled by the last kernel() call when BASS_KERNEL_TRACE=1 (for test.py)
LAST_EXEC_NS = None


def _split(total, hi):
    """Split `total` into near-equal (offset, size) parts, each <= hi and
    a multiple of 8 (last part takes the remainder). Near-equal keeps every
    part's matmul streaming time above the PE sequencer's per-instruction
    issue cost (~302 cols at DoubleRow rate)."""
    n = -(-total // hi)
    base = (total // n) & ~7
    parts = [base] * (n - 1) + [total - base * (n - 1)]
    out = []
    t0 = 0
    for p in parts:
        out.append((t0, p))
        t0 += p
    return out


def _split_greedy(total, hi, lo=256):
    """Greedy split: hi-sized parts, each in [lo, hi]."""
    assert total >= lo
    parts = []
    rem = total
    while rem > hi:
        take = hi if rem - hi >= lo else rem - lo
        parts.append(take)
        rem -= take
    parts.append(rem)
    out = []
    t0 = 0
    for p in parts:
        out.append((t0, p))
        t0 += p
    return out


def _build_program(cap, tiling="equal", w3_ring="sp", xc_mode="hyb2",
                   wchunk="all4", wbufs=2, warmup=0, w2chunk=4, f0i=False,
                   psum_store=False):
    nc = bacc.Bacc("TRN2", target_bir_lowering=False, debug=False,
                   num_devices=N_CORES)

    xc_d = nc.dram_tensor("xc", [KD, 128, 2, cap], FP8, kind="ExternalInput")
    w1l_d = nc.dram_tensor("w1l", [KF, 128, KD, 2, 128], FP8,
                           kind="ExternalInput")
    w3l_d = nc.dram_tensor("w3l", [KF, 128, KD, 2, 128], FP8,
                           kind="ExternalInput")
    w2l_d = nc.dram_tensor("w2l", [KD, 128, KF, 2, 128], FP8,
                           kind="ExternalInput")
    yt_d = nc.dram_tensor("yt", [KD, 128, cap], F32, kind="ExternalOutput")

    silu = mybir.ActivationFunctionType.Silu
    copyf = mybir.ActivationFunctionType.Copy
    mult = mybir.AluOpType.mult
    sub = mybir.AluOpType.subtract
    p1t = _split(cap, 512) if tiling == "equal" else _split_greedy(cap, 512)

    with tile.TileContext(nc) as tc:
        with (
            tc.tile_pool(name="xc", bufs=1) as xpool,
            tc.tile_pool(name="hc", bufs=1) as hpool,
            tc.tile_pool(name="w13", bufs=wbufs) as wpool,
            tc.tile_pool(name="w2", bufs=2) as w2pool,
            tc.tile_pool(name="hst", bufs=3) as spool,
            tc.tile_pool(name="yo", bufs=3) as ypool,
            tc.tile_pool(name="ps1", bufs=3, space="PSUM") as psum1,
            tc.tile_pool(name="ps2", bufs=2, space="PSUM") as psum2,
        ):
            # x (hi+lo fp8) loads on SWDGE (gpsimd ring) — SP and ACT rings
            # carry the w1/w3 weight streams. hi planes stream first: the
            # hi-hi matmul pass only reads plane 0, so the PE can start while
            # the lo planes are still in flight.
            xc_s = xpool.tile([128, KD, 2, cap], FP8)
            if xc_mode == "plane":
                for j in range(2):
                    for k in range(KD):
                        nc.gpsimd.dma_start(xc_s[:, k, j], xc_d[k, :, j])
            elif xc_mode == "spring":
                # first two k-chunks ahead of the f=0 weights on the SP ring;
                # the rest interleave after them (see f-loop below). One ring
                # means the serial DMA mover serves strictly in need-order.
                for k in range(2):
                    nc.sync.dma_start(xc_s[:, k], xc_d[k])
            elif xc_mode == "hyb3":
                # k0,k1 are the ring's first two descriptors; the rest follow
                # in lockstep-consumption order (see f=0 below)
                for k in range(2):
                    nc.sync.dma_start(xc_s[:, k], xc_d[k])
            elif xc_mode == "hyb5":
                pass  # k0,k1 issued inside the f=0 paired order below
            elif xc_mode in ("hyb", "hyb4", "hyb2"):
                # first two k-chunks via SWDGE (cheap descriptors, parallel
                # to the HWDGE ring) so the first matmul starts ~2us earlier;
                # the bulk still streams in need-order on the SP ring.
                for k in range(2):
                    nc.gpsimd.dma_start(xc_s[:, k], xc_d[k])
            else:
                for k in range(KD):
                    nc.gpsimd.dma_start(xc_s[:, k], xc_d[k])

            if warmup:
                # PE p-state pre-warm: ramp the PE clock on a memset tile
                # while the DMA pipes fill (a PE idle gap resets the clock
                # ramp; entering real work at full clock saves the ~2x
                # mid-pstate tax on the first ~3us of matmuls).
                wup = xpool.tile([128, 2, 512], FP8, name="wup")
                nc.vector.memset(wup[:], 0.0)
                wps = psum1.tile([128, 512], F32, tag="h1p")
                for i in range(warmup):
                    nc.tensor.matmul(wps[:], wup[:, :, :128], wup[:],
                                     start=True, stop=True, perf_mode=DR)
            # SBUF-resident two-level fp8 SwiGLU intermediate
            hc = hpool.tile([128, KF, 2, cap], FP8)

            def dr_group(out_ap, wc, xs, nk, t0, tn):
                """3-term compensated GEMM: one PSUM accumulation group of
                nk/2 hi-hi DoubleRow matmuls + nk cross DoubleRow matmuls."""
                for j in range(nk // 2):
                    nc.tensor.matmul(
                        out_ap, wc[:, 2 * j:2 * j + 2, 1, :],
                        xs[:, 2 * j:2 * j + 2, 0, t0:t0 + tn],
                        start=(j == 0), stop=False, perf_mode=DR)
                for k in range(nk):
                    # plane0: W_lo_k x X_hi_k, plane1: W_hi_k x X_lo_k
                    nc.tensor.matmul(
                        out_ap, wc[:, k], xs[:, k, :, t0:t0 + tn],
                        start=False, stop=(k == nk - 1), perf_mode=DR)

            # ---- Phase 1: hT = silu(w1 @ xT) * (w3 @ xT), fp8 in SBUF ----
            for f in range(KF):
                # w1 on the SP ring, w3 on the ACT ring. The first few tiles
                # are chunked so the PE's first matmuls wait on a quarter-tile
                # transfer; beyond that one descriptor per tile minimizes ring
                # sequencing.
                nch = 4 if (wchunk == "all4" or f < 3) else 1
                w3_eng = nc.scalar if w3_ring == "act" else nc.sync
                w1c = wpool.tile([128, KD, 2, 128], FP8, tag="w1c")
                w3c = wpool.tile([128, KD, 2, 128], FP8, tag="w3c")
                if f == 0 and xc_mode in ("hyb2", "hyb3", "hyb5"):
                    # lockstep-consumption DMA order for the interleaved f=0
                    # emission: [w1c_c, w3c_c, xc ks(c)] so the serial ring
                    # delivers operands exactly as the 6 PSUM groups consume
                    # them (k0,k1 arrive via SWDGE)
                    ksets = [(2, 4), (4, 8), (8, 12), (12, KD)]
                    for c in range(4):
                        sl = slice(c * 4, (c + 1) * 4)
                        nc.sync.dma_start(w1c[:, sl], w1l_d[f][:, sl])
                        if c == 0 and xc_mode == "hyb5":
                            nc.sync.dma_start(xc_s[:, 0], xc_d[0])
                            nc.sync.dma_start(xc_s[:, 1], xc_d[1])
                        nc.sync.dma_start(w3c[:, sl], w3l_d[f][:, sl])
                        for k in range(*ksets[c]):
                            nc.sync.dma_start(xc_s[:, k], xc_d[k])
                else:
                    for c in range(nch):
                        sl = slice(c * KD // nch, (c + 1) * KD // nch)
                        nc.sync.dma_start(w1c[:, sl], w1l_d[f][:, sl])
                    for c in range(nch):
                        sl = slice(c * KD // nch, (c + 1) * KD // nch)
                        w3_eng.dma_start(w3c[:, sl], w3l_d[f][:, sl])
                if f == 0 and xc_mode in ("spring", "hyb"):
                    for k in range(2, KD):
                        nc.sync.dma_start(xc_s[:, k], xc_d[k])
                elif f == 0 and xc_mode == "hyb4":
                    # bulk xc as 4 grouped descriptors: fewer ring slots
                    for g in range(4):
                        k0, k1 = 2 + g * 4, min(2 + (g + 1) * 4, KD)
                        nc.sync.dma_start(xc_s[:, k0:k1], xc_d[k0:k1])

                def epilogue(t0, tn, h1p, h3p):
                    s1 = spool.tile([128, 512], F32, tag="s1", name="s1")
                    nc.scalar.activation(s1[:, :tn], h1p[:, :tn], silu,
                                         scale=INV_SXW)
                    ht = spool.tile([128, 512], F32, tag="ht", name="ht")
                    nc.vector.scalar_tensor_tensor(
                        ht[:, :tn], h3p[:, :tn], G3_TO_HT, s1[:, :tn],
                        mult, mult)
                    nc.gpsimd.tensor_copy(hc[:, f, 0, t0:t0 + tn], ht[:, :tn])
                    nc.vector.tensor_tensor(
                        hc[:, f, 1, t0:t0 + tn], ht[:, :tn],
                        hc[:, f, 0, t0:t0 + tn], sub)

                if f0i and f == 0:
                    # f=0 runs while xc is still streaming in k order: emit
                    # all 6 PSUM groups (w1/w3 x 3 token tiles) in k-lockstep
                    # so the PE always has work on already-resident k-chunks.
                    groups = []
                    for (t0, tn) in p1t:
                        h1p = psum1.tile([128, 512], F32, tag="h1p",
                                         name="h1p")
                        h3p = psum1.tile([128, 512], F32, tag="h3p",
                                         name="h3p")
                        groups.append((h1p, w1c, t0, tn))
                        groups.append((h3p, w3c, t0, tn))
                    for j in range(KD // 2):
                        for (pt, wc, t0, tn) in groups:
                            nc.tensor.matmul(
                                pt[:, :tn], wc[:, 2 * j:2 * j + 2, 1, :],
                                xc_s[:, 2 * j:2 * j + 2, 0, t0:t0 + tn],
                                start=(j == 0), stop=False, perf_mode=DR)
                    for k in range(KD):
                        for (pt, wc, t0, tn) in groups:
                            nc.tensor.matmul(
                                pt[:, :tn], wc[:, k],
                                xc_s[:, k, :, t0:t0 + tn],
                                start=False, stop=(k == KD - 1), perf_mode=DR)
                    for i, (t0, tn) in enumerate(p1t):
                        epilogue(t0, tn, groups[2 * i][0], groups[2 * i + 1][0])
                else:
                    for (t0, tn) in p1t:
                        h1p = psum1.tile([128, 512], F32, tag="h1p",
                                         name="h1p")
                        dr_group(h1p[:, :tn], w1c, xc_s, KD, t0, tn)
                        h3p = psum1.tile([128, 512], F32, tag="h3p",
                                         name="h3p")
                        dr_group(h3p[:, :tn], w3c, xc_s, KD, t0, tn)
                        epilogue(t0, tn, h1p, h3p)

            # ---- Phase 2: yT = w2 @ hT ----
            for m in range(KD):
                w2c = w2pool.tile([128, KF, 2, 128], FP8, tag="w2c")
                # chunk-split so the first matmuls of this m start after
                # a fraction of the weight load instead of all of it
                cw = KF // w2chunk
                for c in range(w2chunk):
                    nc.sync.dma_start(w2c[:, c * cw:(c + 1) * cw],
                                      w2l_d[m][:, c * cw:(c + 1) * cw])
                for (t0, tn) in p1t:
                    yp = psum2.tile([128, 512], F32, tag="yp")
                    dr_group(yp[:, :tn], w2c, hc, KF, t0, tn)
                    if psum_store:
                        # store raw y*S_W*S_H; the host folds the 2^-12
                        # dequant into the probs gather — no Act copy on the
                        # critical path
                        nc.scalar.dma_start(yt_d[m][:, t0:t0 + tn],
                                            yp[:, :tn])
                    else:
                        yo = ypool.tile([128, 512], F32, tag="yo")
                        nc.scalar.activation(yo[:, :tn], yp[:, :tn], copyf,
                                             scale=INV_SWH)
                        nc.scalar.dma_start(yt_d[m][:, t0:t0 + tn],
                                            yo[:, :tn])

    nc.compile()
    return nc


def _quant2(a, scale):
    """Two-level e4m3 quantization at a shared power-of-2 scale.
    Returns (hi, lo) fp8 arrays with hi + lo ~= a * scale."""
    s = (a * scale).astype(np.float32)
    hi = s.astype(NP_FP8)
    lo = (s - hi.astype(np.float32)).astype(NP_FP8)
    return hi, lo


def _pack_w(w, scale):
    """[R, C] f32 -> [R//128, 128, C//128, 2, 128] fp8 with
    out[r, p, k, j, m] = Wq_j[r*128+m, k*128+p]; j=0 lo, j=1 hi."""
    R, C = w.shape
    hi, lo = _quant2(w, scale)
    q = np.stack([lo, hi])                       # [2, R, C]
    q = q.reshape(2, R // 128, 128, C // 128, 128)
    return np.ascontiguousarray(q.transpose(1, 4, 3, 0, 2))


def kernel(x, expert_indices, expert_weights, w1, w2, w3):
    global LAST_EXEC_NS
    x = np.ascontiguousarray(np.asarray(x, dtype=np.float32))
    routing = np.asarray(expert_indices)
    probs = np.asarray(expert_weights, dtype=np.float32)
    w1 = np.asarray(w1, dtype=np.float32)
    w2 = np.asarray(w2, dtype=np.float32)
    w3 = np.asarray(w3, dtype=np.float32)
    n_tokens = x.shape[0]

    idxs = [np.flatnonzero(routing[:, e]) for e in range(NUM_EXPERTS)]
    max_count = max(len(i) for i in idxs)
    cap = max(512, -(-max_count // 16) * 16)  # round up to multiple of 16
    assert cap <= 2304, f"unexpectedly imbalanced routing: max_count={max_count}"

    if cap not in _PROGRAMS:
        _PROGRAMS[cap] = _build_program(cap)
    nc = _PROGRAMS[cap]

    def _prep(e):
        idx = idxs[e]
        xt = np.zeros((DIM, cap), dtype=np.float32)
        xt[:, :len(idx)] = x[idx].T
        xhi, xlo = _quant2(xt, S_X)
        xq = np.stack([xhi, xlo])                # [2, DIM, cap]
        xq = xq.reshape(2, KD, 128, cap)
        return {
            "xc": np.ascontiguousarray(xq.transpose(1, 2, 0, 3)),
            "w1l": _pack_w(w1[e], S_W),
            "w3l": _pack_w(w3[e], S_W),
            "w2l": _pack_w(w2[e], S_W),
        }

    from concurrent.futures import ThreadPoolExecutor
    with ThreadPoolExecutor(max_workers=NUM_EXPERTS) as pool:
        in_maps = list(pool.map(_prep, range(NUM_EXPERTS)))

    trace = os.environ.get("BASS_KERNEL_TRACE", "0") == "1"
    if trace:
        import importlib.util
        if importlib.util.find_spec("antenv") is None or importlib.util.find_spec(
                "antenv.axon_hooks") is None:
            trace = False  # NTFF hook unavailable in this environment
    res = run_bass_kernel_spmd(
        nc, in_maps, core_ids=list(range(N_CORES)),
        trace=trace, trace_cores=list(range(N_CORES)) if trace else None,
    )
    LAST_EXEC_NS = res.exec_time_ns

    out = np.zeros((n_tokens, DIM), dtype=np.float32)
    for e in range(NUM_EXPERTS):
        idx = idxs[e]
        y_t = res.results[e]["yt"].reshape(DIM, cap)[:, :len(idx)]
        out[idx] += probs[idx, e][:, None] * y_t.T
    return out


# revision 43
# speedup vs baseline: 1.0089x; 1.0089x over previous
"""Trainium2 Bass kernel for nn_ConditionalFeedForward (MoE top-2 routing).

Strategy: expert-parallel across the 8 NeuronCores — core e owns expert e's
weights. Host (numpy) gathers each expert's routed tokens (multi-hot
routing_map), pads to a common capacity CAP, and pre-quantizes operands to a
two-level fp8-e4m3 representation (hi + lo residual, SAME power-of-2 scale so
all product terms share one PSUM scale). Each core computes, for its expert:

    hT = silu(w1 @ xT) * (w3 @ xT)          # [FFN, CAP], SBUF-resident fp8
    yT = w2 @ hT                            # [DIM, CAP]

Every GEMM runs on the PE in fp8 DoubleRow mode (2 k-chunks per instruction,
0.5 cycles/row = 4x the f32r rate) with 3 compensation terms sharing one PSUM
accumulation group:

    W @ X ~= W_hi@X_hi + W_lo@X_hi + W_hi@X_lo        (drop W_lo@X_lo ~ 1e-3)

The cross terms pair (W_lo_k, X_hi_k) and (W_hi_k, X_lo_k) as the two planes
of a single DoubleRow instruction, so each GEMM costs 3 fp8 K-passes = 0.75x
the one-pass f32r time. End-to-end rel err ~1e-3 (vs 2e-2 budget).

The SwiGLU intermediate h is re-quantized to two-level fp8 on-device
(scalar: silu, vector: scaled-mul + residual-sub, gpsimd: hi cast) and stays
SBUF-resident — no DRAM staging between the two phases. Host scatter-adds
gate-weighted outputs back to the full [N_TOKENS, DIM] result.
"""

import os
import numpy as np
import ml_dtypes

import concourse.bacc as bacc
import concourse.mybir as mybir
import concourse.tile as tile
from concourse.bass_utils import run_bass_kernel_spmd

# Problem constants (hardcoded per harness contract)
NUM_EXPERTS = 8
DIM = 2048
FFN = 5632
N_CORES = 8
KD = DIM // 128    # 16 contraction chunks for GEMM1/3; output chunks for GEMM2
KF = FFN // 128    # 44 ffn chunks

F32 = mybir.dt.float32
FP8 = mybir.dt.float8e4
NP_FP8 = ml_dtypes.float8_e4m3
DR = mybir.MatmulPerfMode.DoubleRow

# Power-of-2 quantization scales. fp8 relative precision is scale-free; the
# scale only positions the distribution inside e4m3's normal range
# (2^-6 .. 240). hi and lo share the scale so all matmul terms accumulate in
# one PSUM group.
S_X = 16.0       # x ~ N(0,1): max|x|*16 ~ 90
S_W = 1024.0     # w ~ 0.02*N(0,1): max|w|*1024 ~ 115
S_H = 4.0        # h = silu(g1)*g3, |h| <~ 20: max|h|*4 ~ 80
INV_SXW = 1.0 / (S_X * S_W)          # PSUM -> g dequant (2^-14)
G3_TO_HT = S_H / (S_X * S_W)         # PSUM g3 -> g3*S_H   (2^-12)
INV_SWH = 1.0 / (S_W * S_H)          # PSUM -> y dequant   (2^-12)

# Compiled program cache keyed by CAP
_PROGRAMS = {}

# Filled by the last kernel() call when BASS_KERNEL_TRACE=1 (for test.py)
LAST_EXEC_NS = None


def _split(total, hi):
    """Split `total` into near-equal (offset, size) parts, each <= hi and
    a multiple of 8 (last part takes the remainder). Near-equal keeps every
    part's matmul streaming time above the PE sequencer's per-instruction
    issue cost (~302 cols at DoubleRow rate)."""
    n = -(-total // hi)
    base = (total // n) & ~7
    parts = [base] * (n - 1) + [total - base * (n - 1)]
    out = []
    t0 = 0
    for p in parts:
        out.append((t0, p))
        t0 += p
    return out


def _split_greedy(total, hi, lo=256):
    """Greedy split: hi-sized parts, each in [lo, hi]."""
    assert total >= lo
    parts = []
    rem = total
    while rem > hi:
        take = hi if rem - hi >= lo else rem - lo
        parts.append(take)
        rem -= take
    parts.append(rem)
    out = []
    t0 = 0
    for p in parts:
        out.append((t0, p))
        t0 += p
    return out


def _build_program(cap, tiling="equal", w3_ring="sp", xc_mode="hyb2",
                   wchunk="all4", wbufs=2, warmup=0, w2chunk=4, f0i=False,
                   psum_store=False):
    nc = bacc.Bacc("TRN2", target_bir_lowering=False, debug=False,
                   num_devices=N_CORES)

    xc_d = nc.dram_tensor("xc", [KD, 128, 2, cap], FP8, kind="ExternalInput")
    w1l_d = nc.dram_tensor("w1l", [KF, 128, KD, 2, 128], FP8,
                           kind="ExternalInput")
    w3l_d = nc.dram_tensor("w3l", [KF, 128, KD, 2, 128], FP8,
                           kind="ExternalInput")
    w2l_d = nc.dram_tensor("w2l", [KD, 128, KF, 2, 128], FP8,
                           kind="ExternalInput")
    yt_d = nc.dram_tensor("yt", [KD, 128, cap], F32, kind="ExternalOutput")

    silu = mybir.ActivationFunctionType.Silu
    copyf = mybir.ActivationFunctionType.Copy
    mult = mybir.AluOpType.mult
    sub = mybir.AluOpType.subtract
    p1t = _split(cap, 512) if tiling == "equal" else _split_greedy(cap, 512)

    with tile.TileContext(nc) as tc:
        with (
            tc.tile_pool(name="xc", bufs=1) as xpool,
            tc.tile_pool(name="hc", bufs=1) as hpool,
            tc.tile_pool(name="w13", bufs=wbufs) as wpool,
            tc.tile_pool(name="w2", bufs=2) as w2pool,
            tc.tile_pool(name="hst", bufs=3) as spool,
            tc.tile_pool(name="yo", bufs=3) as ypool,
            tc.tile_pool(name="ps1", bufs=3, space="PSUM") as psum1,
            tc.tile_pool(name="ps2", bufs=2, space="PSUM") as psum2,
        ):
            # x (hi+lo fp8) loads on SWDGE (gpsimd ring) — SP and ACT rings
            # carry the w1/w3 weight streams. hi planes stream first: the
            # hi-hi matmul pass only reads plane 0, so the PE can start while
            # the lo planes are still in flight.
            xc_s = xpool.tile([128, KD, 2, cap], FP8)
            if xc_mode == "plane":
                for j in range(2):
                    for k in range(KD):
                        nc.gpsimd.dma_start(xc_s[:, k, j], xc_d[k, :, j])
            elif xc_mode == "spring":
                # first two k-chunks ahead of the f=0 weights on the SP ring;
                # the rest interleave after them (see f-loop below). One ring
                # means the serial DMA mover serves strictly in need-order.
                for k in range(2):
                    nc.sync.dma_start(xc_s[:, k], xc_d[k])
            elif xc_mode == "hyb3":
                # k0,k1 are the ring's first two descriptors; the rest follow
                # in lockstep-consumption order (see f=0 below)
                for k in range(2):
                    nc.sync.dma_start(xc_s[:, k], xc_d[k])
            elif xc_mode == "hyb5":
                pass  # k0,k1 issued inside the f=0 paired order below
            elif xc_mode in ("hyb", "hyb4", "hyb2"):
                # first two k-chunks via SWDGE (cheap descriptors, parallel
                # to the HWDGE ring) so the first matmul starts ~2us earlier;
                # the bulk still streams in need-order on the SP ring.
                for k in range(2):
                    nc.gpsimd.dma_start(xc_s[:, k], xc_d[k])
            else:
                for k in range(KD):
                    nc.gpsimd.dma_start(xc_s[:, k], xc_d[k])

            if warmup:
                # PE p-state pre-warm: ramp the PE clock on a memset tile
                # while the DMA pipes fill (a PE idle gap resets the clock
                # ramp; entering real work at full clock saves the ~2x
                # mid-pstate tax on the first ~3us of matmuls).
                wup = xpool.tile([128, 2, 512], FP8, name="wup")
                nc.vector.memset(wup[:], 0.0)
                wps = psum1.tile([128, 512], F32, tag="h1p")
                for i in range(warmup):
                    nc.tensor.matmul(wps[:], wup[:, :, :128], wup[:],
                                     start=True, stop=True, perf_mode=DR)
            # SBUF-resident two-level fp8 SwiGLU intermediate
            hc = hpool.tile([128, KF, 2, cap], FP8)

            def dr_group(out_ap, wc, xs, nk, t0, tn):
                """3-term compensated GEMM: one PSUM accumulation group of
                nk/2 hi-hi DoubleRow matmuls + nk cross DoubleRow matmuls."""
                for j in range(nk // 2):
                    nc.tensor.matmul(
                        out_ap, wc[:, 2 * j:2 * j + 2, 1, :],
                        xs[:, 2 * j:2 * j + 2, 0, t0:t0 + tn],
                        start=(j == 0), stop=False, perf_mode=DR)
                for k in range(nk):
                    # plane0: W_lo_k x X_hi_k, plane1: W_hi_k x X_lo_k
                    nc.tensor.matmul(
                        out_ap, wc[:, k], xs[:, k, :, t0:t0 + tn],
                        start=False, stop=(k == nk - 1), perf_mode=DR)

            # ---- Phase 1: hT = silu(w1 @ xT) * (w3 @ xT), fp8 in SBUF ----
            for f in range(KF):
                # w1 on the SP ring, w3 on the ACT ring. The first few tiles
                # are chunked so the PE's first matmuls wait on a quarter-tile
                # transfer; beyond that one descriptor per tile minimizes ring
                # sequencing.
                nch = 4 if (wchunk == "all4" or f < 3) else 1
                w3_eng = nc.scalar if w3_ring == "act" else nc.sync
                w1c = wpool.tile([128, KD, 2, 128], FP8, tag="w1c")
                w3c = wpool.tile([128, KD, 2, 128], FP8, tag="w3c")
                if f == 0 and xc_mode in ("hyb2", "hyb3", "hyb5"):
                    # lockstep-consumption DMA order for the interleaved f=0
                    # emission: [w1c_c, w3c_c, xc ks(c)] so the serial ring
                    # delivers operands exactly as the 6 PSUM groups consume
                    # them (k0,k1 arrive via SWDGE)
                    ksets = [(2, 4), (4, 8), (8, 12), (12, KD)]
                    for c in range(4):
                        sl = slice(c * 4, (c + 1) * 4)
                        nc.sync.dma_start(w1c[:, sl], w1l_d[f][:, sl])
                        if c == 0 and xc_mode == "hyb5":
                            nc.sync.dma_start(xc_s[:, 0], xc_d[0])
                            nc.sync.dma_start(xc_s[:, 1], xc_d[1])
                        nc.sync.dma_start(w3c[:, sl], w3l_d[f][:, sl])
                        for k in range(*ksets[c]):
                            nc.sync.dma_start(xc_s[:, k], xc_d[k])
                else:
                    for c in range(nch):
                        sl = slice(c * KD // nch, (c + 1) * KD // nch)
                        nc.sync.dma_start(w1c[:, sl], w1l_d[f][:, sl])
                    for c in range(nch):
                        sl = slice(c * KD // nch, (c + 1) * KD // nch)
                        w3_eng.dma_start(w3c[:, sl], w3l_d[f][:, sl])
                if f == 0 and xc_mode in ("spring", "hyb"):
                    for k in range(2, KD):
                        nc.sync.dma_start(xc_s[:, k], xc_d[k])
                elif f == 0 and xc_mode == "hyb4":
                    # bulk xc as 4 grouped descriptors: fewer ring slots
                    for g in range(4):
                        k0, k1 = 2 + g * 4, min(2 + (g + 1) * 4, KD)
                        nc.sync.dma_start(xc_s[:, k0:k1], xc_d[k0:k1])

                def epilogue(t0, tn, h1p, h3p):
                    s1 = spool.tile([128, 512], F32, tag="s1", name="s1")
                    nc.scalar.activation(s1[:, :tn], h1p[:, :tn], silu,
                                         scale=INV_SXW)
                    ht = spool.tile([128, 512], F32, tag="ht", name="ht")
                    nc.vector.scalar_tensor_tensor(
                        ht[:, :tn], h3p[:, :tn], G3_TO_HT, s1[:, :tn],
                        mult, mult)
                    nc.gpsimd.tensor_copy(hc[:, f, 0, t0:t0 + tn], ht[:, :tn])
                    nc.vector.tensor_tensor(
                        hc[:, f, 1, t0:t0 + tn], ht[:, :tn],
                        hc[:, f, 0, t0:t0 + tn], sub)

                if f0i and f == 0:
                    # f=0 runs while xc is still streaming in k order: emit
                    # all 6 PSUM groups (w1/w3 x 3 token tiles) in k-lockstep
                    # so the PE always has work on already-resident k-chunks.
                    groups = []
                    for (t0, tn) in p1t:
                        h1p = psum1.tile([128, 512], F32, tag="h1p",
                                         name="h1p")
                        h3p = psum1.tile([128, 512], F32, tag="h3p",
                                         name="h3p")
                        groups.append((h1p, w1c, t0, tn))
                        groups.append((h3p, w3c, t0, tn))
                    for j in range(KD // 2):
                        for (pt, wc, t0, tn) in groups:
                            nc.tensor.matmul(
                                pt[:, :tn], wc[:, 2 * j:2 * j + 2, 1, :],
                                xc_s[:, 2 * j:2 * j + 2, 0, t0:t0 + tn],
                                start=(j == 0), stop=False, perf_mode=DR)
                    for k in range(KD):
                        for (pt, wc, t0, tn) in groups:
                            nc.tensor.matmul(
                                pt[:, :tn], wc[:, k],
                                xc_s[:, k, :, t0:t0 + tn],
                                start=False, stop=(k == KD - 1), perf_mode=DR)
                    for i, (t0, tn) in enumerate(p1t):
                        epilogue(t0, tn, groups[2 * i][0], groups[2 * i + 1][0])
                else:
                    for (t0, tn) in p1t:
                        h1p = psum1.tile([128, 512], F32, tag="h1p",
                                         name="h1p")
                        dr_group(h1p[:, :tn], w1c, xc_s, KD, t0, tn)
                        h3p = psum1.tile([128, 512], F32, tag="h3p",
                                         name="h3p")
                        dr_group(h3p[:, :tn], w3c, xc_s, KD, t0, tn)
                        epilogue(t0, tn, h1p, h3p)

            # ---- Phase 2: yT = w2 @ hT ----
            for m in range(KD):
                w2c = w2pool.tile([128, KF, 2, 128], FP8, tag="w2c")
                # chunk-split so the first matmuls of this m start after
                # a fraction of the weight load instead of all of it
                cw = KF // w2chunk
                for c in range(w2chunk):
                    nc.sync.dma_start(w2c[:, c * cw:(c + 1) * cw],
                                      w2l_d[m][:, c * cw:(c + 1) * cw])
                for (t0, tn) in p1t:
                    yp = psum2.tile([128, 512], F32, tag="yp")
                    dr_group(yp[:, :tn], w2c, hc, KF, t0, tn)
                    if psum_store:
                        # store raw y*S_W*S_H; the host folds the 2^-12
                        # dequant into the probs gather — no Act copy on the
                        # critical path
                        nc.scalar.dma_start(yt_d[m][:, t0:t0 + tn],
                                            yp[:, :tn])
                    else:
                        yo = ypool.tile([128, 512], F32, tag="yo")
                        nc.scalar.activation(yo[:, :tn], yp[:, :tn], copyf,
                                             scale=INV_SWH)
                        nc.scalar.dma_start(yt_d[m][:, t0:t0 + tn],
                                            yo[:, :tn])

    nc.compile()
    return nc


def _quant2(a, scale):
    """Two-level e4m3 quantization at a shared power-of-2 scale.
    Returns (hi, lo) fp8 arrays with hi + lo ~= a * scale."""
    s = (a * scale).astype(np.float32)
    hi = s.astype(NP_FP8)
    lo = (s - hi.astype(np.float32)).astype(NP_FP8)
    return hi, lo


def _pack_w(w, scale):
    """[R, C] f32 -> [R//128, 128, C//128, 2, 128] fp8 with
    out[r, p, k, j, m] = Wq_j[r*128+m, k*128+p]; j=0 lo, j=1 hi."""
    R, C = w.shape
    hi, lo = _quant2(w, scale)
    q = np.stack([lo, hi])                       # [2, R, C]
    q = q.reshape(2, R // 128, 128, C // 128, 128)
    return np.ascontiguousarray(q.transpose(1, 4, 3, 0, 2))


def kernel(x, expert_indices, expert_weights, w1, w2, w3):
    global LAST_EXEC_NS
    x = np.ascontiguousarray(np.asarray(x, dtype=np.float32))
    routing = np.asarray(expert_indices)
    probs = np.asarray(expert_weights, dtype=np.float32)
    w1 = np.asarray(w1, dtype=np.float32)
    w2 = np.asarray(w2, dtype=np.float32)
    w3 = np.asarray(w3, dtype=np.float32)
    n_tokens = x.shape[0]

    idxs = [np.flatnonzero(routing[:, e]) for e in range(NUM_EXPERTS)]
    max_count = max(len(i) for i in idxs)
    # capacity = max expert load, rounded to 8 (device requires alignment;
    # r4 hit NRT_EXEC_UNIT_UNRECOVERABLE; PE time scales linearly with cap)
    cap = max(512, -(-max_count // 8) * 8)
    assert cap <= 2304, f"unexpectedly imbalanced routing: max_count={max_count}"

    if cap not in _PROGRAMS:
        _PROGRAMS[cap] = _build_program(cap)
    nc = _PROGRAMS[cap]

    def _prep(e):
        idx = idxs[e]
        xt = np.zeros((DIM, cap), dtype=np.float32)
        xt[:, :len(idx)] = x[idx].T
        xhi, xlo = _quant2(xt, S_X)
        xq = np.stack([xhi, xlo])                # [2, DIM, cap]
        xq = xq.reshape(2, KD, 128, cap)
        return {
            "xc": np.ascontiguousarray(xq.transpose(1, 2, 0, 3)),
            "w1l": _pack_w(w1[e], S_W),
            "w3l": _pack_w(w3[e], S_W),
            "w2l": _pack_w(w2[e], S_W),
        }

    from concurrent.futures import ThreadPoolExecutor
    with ThreadPoolExecutor(max_workers=NUM_EXPERTS) as pool:
        in_maps = list(pool.map(_prep, range(NUM_EXPERTS)))

    trace = os.environ.get("BASS_KERNEL_TRACE", "0") == "1"
    if trace:
        import importlib.util
        if importlib.util.find_spec("antenv") is None or importlib.util.find_spec(
                "antenv.axon_hooks") is None:
            trace = False  # NTFF hook unavailable in this environment
    res = run_bass_kernel_spmd(
        nc, in_maps, core_ids=list(range(N_CORES)),
        trace=trace, trace_cores=list(range(N_CORES)) if trace else None,
    )
    LAST_EXEC_NS = res.exec_time_ns

    out = np.zeros((n_tokens, DIM), dtype=np.float32)
    for e in range(NUM_EXPERTS):
        idx = idxs[e]
        y_t = res.results[e]["yt"].reshape(DIM, cap)[:, :len(idx)]
        out[idx] += probs[idx, e][:, None] * y_t.T
    return out


# revision 51
# speedup vs baseline: 1.0168x; 1.0078x over previous
"""Trainium2 Bass kernel for nn_ConditionalFeedForward (MoE top-2 routing).

Strategy: expert-parallel across the 8 NeuronCores — core e owns expert e's
weights. Host (numpy) gathers each expert's routed tokens (multi-hot
routing_map), pads to a common capacity CAP, and pre-quantizes operands to a
two-level fp8-e4m3 representation (hi + lo residual, SAME power-of-2 scale so
all product terms share one PSUM scale). Each core computes, for its expert:

    hT = silu(w1 @ xT) * (w3 @ xT)          # [FFN, CAP], SBUF-resident fp8
    yT = w2 @ hT                            # [DIM, CAP]

Every GEMM runs on the PE in fp8 DoubleRow mode (2 k-chunks per instruction,
0.5 cycles/row = 4x the f32r rate) with 3 compensation terms sharing one PSUM
accumulation group:

    W @ X ~= W_hi@X_hi + W_lo@X_hi + W_hi@X_lo        (drop W_lo@X_lo ~ 1e-3)

The cross terms pair (W_lo_k, X_hi_k) and (W_hi_k, X_lo_k) as the two planes
of a single DoubleRow instruction, so each GEMM costs 3 fp8 K-passes = 0.75x
the one-pass f32r time. End-to-end rel err ~1e-3 (vs 2e-2 budget).

The SwiGLU intermediate h is re-quantized to two-level fp8 on-device
(scalar: silu, vector: scaled-mul + residual-sub, gpsimd: hi cast) and stays
SBUF-resident — no DRAM staging between the two phases. Host scatter-adds
gate-weighted outputs back to the full [N_TOKENS, DIM] result.
"""

import os
import numpy as np
import ml_dtypes

import concourse.bacc as bacc
import concourse.mybir as mybir
import concourse.tile as tile
from concourse.bass_utils import run_bass_kernel_spmd

# Problem constants (hardcoded per harness contract)
NUM_EXPERTS = 8
DIM = 2048
FFN = 5632
N_CORES = 8
KD = DIM // 128    # 16 contraction chunks for GEMM1/3; output chunks for GEMM2
KF = FFN // 128    # 44 ffn chunks

F32 = mybir.dt.float32
FP8 = mybir.dt.float8e4
NP_FP8 = ml_dtypes.float8_e4m3
DR = mybir.MatmulPerfMode.DoubleRow

# Power-of-2 quantization scales. fp8 relative precision is scale-free; the
# scale only positions the distribution inside e4m3's normal range
# (2^-6 .. 240). hi and lo share the scale so all matmul terms accumulate in
# one PSUM group.
S_X = 16.0       # x ~ N(0,1): max|x|*16 ~ 90
S_W = 1024.0     # w ~ 0.02*N(0,1): max|w|*1024 ~ 115
S_H = 4.0        # h = silu(g1)*g3, |h| <~ 20: max|h|*4 ~ 80
INV_SXW = 1.0 / (S_X * S_W)          # PSUM -> g dequant (2^-14)
G3_TO_HT = S_H / (S_X * S_W)         # PSUM g3 -> g3*S_H   (2^-12)
INV_SWH = 1.0 / (S_W * S_H)          # PSUM -> y dequant   (2^-12)

# Compiled program cache keyed by CAP
_PROGRAMS = {}

# Filled by the last kernel() call when BASS_KERNEL_TRACE=1 (for test.py)
LAST_EXEC_NS = None


def _split(total, hi):
    """Split `total` into near-equal (offset, size) parts, each <= hi and
    a multiple of 8 (last part takes the remainder). Near-equal keeps every
    part's matmul streaming time above the PE sequencer's per-instruction
    issue cost (~302 cols at DoubleRow rate)."""
    n = -(-total // hi)
    base = (total // n) & ~7
    parts = [base] * (n - 1) + [total - base * (n - 1)]
    out = []
    t0 = 0
    for p in parts:
        out.append((t0, p))
        t0 += p
    return out


def _split_greedy(total, hi, lo=256):
    """Greedy split: hi-sized parts, each in [lo, hi]."""
    assert total >= lo
    parts = []
    rem = total
    while rem > hi:
        take = hi if rem - hi >= lo else rem - lo
        parts.append(take)
        rem -= take
    parts.append(rem)
    out = []
    t0 = 0
    for p in parts:
        out.append((t0, p))
        t0 += p
    return out


def _build_program(cap, tiling="equal", w3_ring="sp", xc_mode="hyb2",
                   wchunk="all4", wbufs=2, warmup=0, w2chunk=4, f0i="x",
                   psum_store=False):
    nc = bacc.Bacc("TRN2", target_bir_lowering=False, debug=False,
                   num_devices=N_CORES)

    xc_d = nc.dram_tensor("xc", [KD, 128, 2, cap], FP8, kind="ExternalInput")
    w1l_d = nc.dram_tensor("w1l", [KF, 128, KD, 2, 128], FP8,
                           kind="ExternalInput")
    w3l_d = nc.dram_tensor("w3l", [KF, 128, KD, 2, 128], FP8,
                           kind="ExternalInput")
    w2l_d = nc.dram_tensor("w2l", [KD, 128, KF, 2, 128], FP8,
                           kind="ExternalInput")
    yt_d = nc.dram_tensor("yt", [KD, 128, cap], F32, kind="ExternalOutput")

    silu = mybir.ActivationFunctionType.Silu
    copyf = mybir.ActivationFunctionType.Copy
    mult = mybir.AluOpType.mult
    sub = mybir.AluOpType.subtract
    p1t = _split(cap, 512) if tiling == "equal" else _split_greedy(cap, 512)

    with tile.TileContext(nc) as tc:
        with (
            tc.tile_pool(name="xc", bufs=1) as xpool,
            tc.tile_pool(name="hc", bufs=1) as hpool,
            tc.tile_pool(name="w13", bufs=wbufs) as wpool,
            tc.tile_pool(name="w2", bufs=2) as w2pool,
            tc.tile_pool(name="hst", bufs=3) as spool,
            tc.tile_pool(name="yo", bufs=3) as ypool,
            tc.tile_pool(name="ps1", bufs=3, space="PSUM") as psum1,
            tc.tile_pool(name="ps2", bufs=2, space="PSUM") as psum2,
        ):
            # x (hi+lo fp8) loads on SWDGE (gpsimd ring) — SP and ACT rings
            # carry the w1/w3 weight streams. hi planes stream first: the
            # hi-hi matmul pass only reads plane 0, so the PE can start while
            # the lo planes are still in flight.
            xc_s = xpool.tile([128, KD, 2, cap], FP8)
            if xc_mode == "plane":
                for j in range(2):
                    for k in range(KD):
                        nc.gpsimd.dma_start(xc_s[:, k, j], xc_d[k, :, j])
            elif xc_mode == "spring":
                # first two k-chunks ahead of the f=0 weights on the SP ring;
                # the rest interleave after them (see f-loop below). One ring
                # means the serial DMA mover serves strictly in need-order.
                for k in range(2):
                    nc.sync.dma_start(xc_s[:, k], xc_d[k])
            elif xc_mode == "hyb3":
                # k0,k1 are the ring's first two descriptors; the rest follow
                # in lockstep-consumption order (see f=0 below)
                for k in range(2):
                    nc.sync.dma_start(xc_s[:, k], xc_d[k])
            elif xc_mode == "hyb5":
                pass  # k0,k1 issued inside the f=0 paired order below
            elif xc_mode == "hyb7":
                # k0 via SWDGE; k1 becomes ring descriptor #2 (f=0 below)
                nc.gpsimd.dma_start(xc_s[:, 0], xc_d[0])
            elif xc_mode in ("hyb", "hyb4", "hyb2"):
                # first two k-chunks via SWDGE (cheap descriptors, parallel
                # to the HWDGE ring) so the first matmul starts ~2us earlier;
                # the bulk still streams in need-order on the SP ring.
                for k in range(2):
                    nc.gpsimd.dma_start(xc_s[:, k], xc_d[k])
            else:
                for k in range(KD):
                    nc.gpsimd.dma_start(xc_s[:, k], xc_d[k])

            if warmup:
                # PE p-state pre-warm: ramp the PE clock on a memset tile
                # while the DMA pipes fill (a PE idle gap resets the clock
                # ramp; entering real work at full clock saves the ~2x
                # mid-pstate tax on the first ~3us of matmuls).
                wup = xpool.tile([128, 2, 512], FP8, name="wup")
                nc.vector.memset(wup[:], 0.0)
                wps = psum1.tile([128, 512], F32, tag="h1p")
                for i in range(warmup):
                    nc.tensor.matmul(wps[:], wup[:, :, :128], wup[:],
                                     start=True, stop=True, perf_mode=DR)
            # SBUF-resident two-level fp8 SwiGLU intermediate
            hc = hpool.tile([128, KF, 2, cap], FP8)

            def dr_group(out_ap, wc, xs, nk, t0, tn, xfirst=False):
                """3-term compensated GEMM: one PSUM accumulation group of
                nk/2 hi-hi DoubleRow matmuls + nk cross DoubleRow matmuls.
                xfirst emits the k0/k1 cross matmuls first (each needs one
                x chunk, not a pair) so the PE can start one chunk earlier
                during the cold-start x stream."""
                if xfirst:
                    for k in range(2):
                        nc.tensor.matmul(
                            out_ap, wc[:, k], xs[:, k, :, t0:t0 + tn],
                            start=(k == 0), stop=False, perf_mode=DR)
                for j in range(nk // 2):
                    nc.tensor.matmul(
                        out_ap, wc[:, 2 * j:2 * j + 2, 1, :],
                        xs[:, 2 * j:2 * j + 2, 0, t0:t0 + tn],
                        start=(j == 0 and not xfirst), stop=False,
                        perf_mode=DR)
                for k in range(2 if xfirst else 0, nk):
                    # plane0: W_lo_k x X_hi_k, plane1: W_hi_k x X_lo_k
                    nc.tensor.matmul(
                        out_ap, wc[:, k], xs[:, k, :, t0:t0 + tn],
                        start=False, stop=(k == nk - 1), perf_mode=DR)

            # ---- Phase 1: hT = silu(w1 @ xT) * (w3 @ xT), fp8 in SBUF ----
            for f in range(KF):
                # w1 on the SP ring, w3 on the ACT ring. The first few tiles
                # are chunked so the PE's first matmuls wait on a quarter-tile
                # transfer; beyond that one descriptor per tile minimizes ring
                # sequencing.
                nch = 4 if (wchunk == "all4" or f < 3) else 1
                w3_eng = nc.scalar if w3_ring == "act" else nc.sync
                w1c = wpool.tile([128, KD, 2, 128], FP8, tag="w1c")
                w3c = wpool.tile([128, KD, 2, 128], FP8, tag="w3c")
                if f == 0 and xc_mode in ("hyb2", "hyb3", "hyb5", "hyb7"):
                    # lockstep-consumption DMA order for the interleaved f=0
                    # emission: [w1c_c, w3c_c, xc ks(c)] so the serial ring
                    # delivers operands exactly as the 6 PSUM groups consume
                    # them (k0,k1 arrive via SWDGE)
                    ksets = [(2, 4), (4, 8), (8, 12), (12, KD)]
                    for c in range(4):
                        sl = slice(c * 4, (c + 1) * 4)
                        nc.sync.dma_start(w1c[:, sl], w1l_d[f][:, sl])
                        if c == 0 and xc_mode == "hyb5":
                            nc.sync.dma_start(xc_s[:, 0], xc_d[0])
                            nc.sync.dma_start(xc_s[:, 1], xc_d[1])
                        elif c == 0 and xc_mode == "hyb7":
                            nc.sync.dma_start(xc_s[:, 1], xc_d[1])
                        nc.sync.dma_start(w3c[:, sl], w3l_d[f][:, sl])
                        for k in range(*ksets[c]):
                            nc.sync.dma_start(xc_s[:, k], xc_d[k])
                else:
                    for c in range(nch):
                        sl = slice(c * KD // nch, (c + 1) * KD // nch)
                        nc.sync.dma_start(w1c[:, sl], w1l_d[f][:, sl])
                    for c in range(nch):
                        sl = slice(c * KD // nch, (c + 1) * KD // nch)
                        w3_eng.dma_start(w3c[:, sl], w3l_d[f][:, sl])
                if f == 0 and xc_mode in ("spring", "hyb"):
                    for k in range(2, KD):
                        nc.sync.dma_start(xc_s[:, k], xc_d[k])
                elif f == 0 and xc_mode == "hyb4":
                    # bulk xc as 4 grouped descriptors: fewer ring slots
                    for g in range(4):
                        k0, k1 = 2 + g * 4, min(2 + (g + 1) * 4, KD)
                        nc.sync.dma_start(xc_s[:, k0:k1], xc_d[k0:k1])

                def epilogue(t0, tn, h1p, h3p):
                    s1 = spool.tile([128, 512], F32, tag="s1", name="s1")
                    nc.scalar.activation(s1[:, :tn], h1p[:, :tn], silu,
                                         scale=INV_SXW)
                    ht = spool.tile([128, 512], F32, tag="ht", name="ht")
                    nc.vector.scalar_tensor_tensor(
                        ht[:, :tn], h3p[:, :tn], G3_TO_HT, s1[:, :tn],
                        mult, mult)
                    nc.gpsimd.tensor_copy(hc[:, f, 0, t0:t0 + tn], ht[:, :tn])
                    nc.vector.tensor_tensor(
                        hc[:, f, 1, t0:t0 + tn], ht[:, :tn],
                        hc[:, f, 0, t0:t0 + tn], sub)

                xfirst = (f == 0)
                if f0i == "x" and f == 0:
                    # k-lockstep across all 6 PSUM groups with hi-hi pairs
                    # interleaved after each odd k: consumption (~12 instrs
                    # per 2 chunks) pipelines against the serial DMA mover's
                    # per-chunk delivery, so the PE never stalls on the
                    # cold-start x stream.
                    groups = []
                    for (t0, tn) in p1t:
                        h1p = psum1.tile([128, 512], F32, tag="h1p",
                                         name="h1p")
                        h3p = psum1.tile([128, 512], F32, tag="h3p",
                                         name="h3p")
                        groups.append((h1p, w1c, t0, tn))
                        groups.append((h3p, w3c, t0, tn))
                    for k in range(KD):
                        for (pt, wc, t0, tn) in groups:
                            nc.tensor.matmul(
                                pt[:, :tn], wc[:, k],
                                xc_s[:, k, :, t0:t0 + tn],
                                start=(k == 0), stop=False, perf_mode=DR)
                        if k % 2 == 1:
                            j = k // 2
                            for (pt, wc, t0, tn) in groups:
                                nc.tensor.matmul(
                                    pt[:, :tn], wc[:, 2 * j:2 * j + 2, 1, :],
                                    xc_s[:, 2 * j:2 * j + 2, 0, t0:t0 + tn],
                                    start=False, stop=(k == KD - 1),
                                    perf_mode=DR)
                    for i, (t0, tn) in enumerate(p1t):
                        epilogue(t0, tn, groups[2 * i][0], groups[2 * i + 1][0])
                elif f0i and f == 0:
                    # f=0 runs while xc is still streaming in k order: emit
                    # all 6 PSUM groups (w1/w3 x 3 token tiles) in k-lockstep
                    # so the PE always has work on already-resident k-chunks.
                    groups = []
                    for (t0, tn) in p1t:
                        h1p = psum1.tile([128, 512], F32, tag="h1p",
                                         name="h1p")
                        h3p = psum1.tile([128, 512], F32, tag="h3p",
                                         name="h3p")
                        groups.append((h1p, w1c, t0, tn))
                        groups.append((h3p, w3c, t0, tn))
                    for j in range(KD // 2):
                        for (pt, wc, t0, tn) in groups:
                            nc.tensor.matmul(
                                pt[:, :tn], wc[:, 2 * j:2 * j + 2, 1, :],
                                xc_s[:, 2 * j:2 * j + 2, 0, t0:t0 + tn],
                                start=(j == 0), stop=False, perf_mode=DR)
                    for k in range(KD):
                        for (pt, wc, t0, tn) in groups:
                            nc.tensor.matmul(
                                pt[:, :tn], wc[:, k],
                                xc_s[:, k, :, t0:t0 + tn],
                                start=False, stop=(k == KD - 1), perf_mode=DR)
                    for i, (t0, tn) in enumerate(p1t):
                        epilogue(t0, tn, groups[2 * i][0], groups[2 * i + 1][0])
                else:
                    for (t0, tn) in p1t:
                        h1p = psum1.tile([128, 512], F32, tag="h1p",
                                         name="h1p")
                        dr_group(h1p[:, :tn], w1c, xc_s, KD, t0, tn,
                                 xfirst=xfirst)
                        h3p = psum1.tile([128, 512], F32, tag="h3p",
                                         name="h3p")
                        dr_group(h3p[:, :tn], w3c, xc_s, KD, t0, tn,
                                 xfirst=xfirst)
                        epilogue(t0, tn, h1p, h3p)

            # ---- Phase 2: yT = w2 @ hT ----
            for m in range(KD):
                w2c = w2pool.tile([128, KF, 2, 128], FP8, tag="w2c")
                # chunk-split so the first matmuls of this m start after
                # a fraction of the weight load instead of all of it
                cw = KF // w2chunk
                for c in range(w2chunk):
                    nc.sync.dma_start(w2c[:, c * cw:(c + 1) * cw],
                                      w2l_d[m][:, c * cw:(c + 1) * cw])
                for (t0, tn) in p1t:
                    yp = psum2.tile([128, 512], F32, tag="yp")
                    dr_group(yp[:, :tn], w2c, hc, KF, t0, tn)
                    if psum_store:
                        # store raw y*S_W*S_H; the host folds the 2^-12
                        # dequant into the probs gather — no Act copy on the
                        # critical path
                        nc.scalar.dma_start(yt_d[m][:, t0:t0 + tn],
                                            yp[:, :tn])
                    else:
                        yo = ypool.tile([128, 512], F32, tag="yo")
                        nc.scalar.activation(yo[:, :tn], yp[:, :tn], copyf,
                                             scale=INV_SWH)
                        nc.scalar.dma_start(yt_d[m][:, t0:t0 + tn],
                                            yo[:, :tn])

    nc.compile()
    return nc


def _quant2(a, scale):
    """Two-level e4m3 quantization at a shared power-of-2 scale.
    Returns (hi, lo) fp8 arrays with hi + lo ~= a * scale."""
    s = (a * scale).astype(np.float32)
    hi = s.astype(NP_FP8)
    lo = (s - hi.astype(np.float32)).astype(NP_FP8)
    return hi, lo


def _pack_w(w, scale):
    """[R, C] f32 -> [R//128, 128, C//128, 2, 128] fp8 with
    out[r, p, k, j, m] = Wq_j[r*128+m, k*128+p]; j=0 lo, j=1 hi."""
    R, C = w.shape
    hi, lo = _quant2(w, scale)
    q = np.stack([lo, hi])                       # [2, R, C]
    q = q.reshape(2, R // 128, 128, C // 128, 128)
    return np.ascontiguousarray(q.transpose(1, 4, 3, 0, 2))


def kernel(x, expert_indices, expert_weights, w1, w2, w3):
    global LAST_EXEC_NS
    x = np.ascontiguousarray(np.asarray(x, dtype=np.float32))
    routing = np.asarray(expert_indices)
    probs = np.asarray(expert_weights, dtype=np.float32)
    w1 = np.asarray(w1, dtype=np.float32)
    w2 = np.asarray(w2, dtype=np.float32)
    w3 = np.asarray(w3, dtype=np.float32)
    n_tokens = x.shape[0]

    idxs = [np.flatnonzero(routing[:, e]) for e in range(NUM_EXPERTS)]
    max_count = max(len(i) for i in idxs)
    # capacity = max expert load, rounded to 8 (device requires alignment;
    # r4 hit NRT_EXEC_UNIT_UNRECOVERABLE; PE time scales linearly with cap)
    cap = max(512, -(-max_count // 8) * 8)
    assert cap <= 2304, f"unexpectedly imbalanced routing: max_count={max_count}"

    if cap not in _PROGRAMS:
        _PROGRAMS[cap] = _build_program(cap)
    nc = _PROGRAMS[cap]

    def _prep(e):
        idx = idxs[e]
        xt = np.zeros((DIM, cap), dtype=np.float32)
        xt[:, :len(idx)] = x[idx].T
        xhi, xlo = _quant2(xt, S_X)
        xq = np.stack([xhi, xlo])                # [2, DIM, cap]
        xq = xq.reshape(2, KD, 128, cap)
        return {
            "xc": np.ascontiguousarray(xq.transpose(1, 2, 0, 3)),
            "w1l": _pack_w(w1[e], S_W),
            "w3l": _pack_w(w3[e], S_W),
            "w2l": _pack_w(w2[e], S_W),
        }

    from concurrent.futures import ThreadPoolExecutor
    with ThreadPoolExecutor(max_workers=NUM_EXPERTS) as pool:
        in_maps = list(pool.map(_prep, range(NUM_EXPERTS)))

    trace = os.environ.get("BASS_KERNEL_TRACE", "0") == "1"
    if trace:
        import importlib.util
        if importlib.util.find_spec("antenv") is None or importlib.util.find_spec(
                "antenv.axon_hooks") is None:
            trace = False  # NTFF hook unavailable in this environment
    res = run_bass_kernel_spmd(
        nc, in_maps, core_ids=list(range(N_CORES)),
        trace=trace, trace_cores=list(range(N_CORES)) if trace else None,
    )
    LAST_EXEC_NS = res.exec_time_ns

    out = np.zeros((n_tokens, DIM), dtype=np.float32)
    for e in range(NUM_EXPERTS):
        idx = idxs[e]
        y_t = res.results[e]["yt"].reshape(DIM, cap)[:, :len(idx)]
        out[idx] += probs[idx, e][:, None] * y_t.T
    return out


# revision 54
# speedup vs baseline: 1.0169x; 1.0001x over previous
"""Trainium2 Bass kernel for nn_ConditionalFeedForward (MoE top-2 routing).

Strategy: expert-parallel across the 8 NeuronCores — core e owns expert e's
weights. Host (numpy) gathers each expert's routed tokens (multi-hot
routing_map), pads to a common capacity CAP, and pre-quantizes operands to a
two-level fp8-e4m3 representation (hi + lo residual, SAME power-of-2 scale so
all product terms share one PSUM scale). Each core computes, for its expert:

    hT = silu(w1 @ xT) * (w3 @ xT)          # [FFN, CAP], SBUF-resident fp8
    yT = w2 @ hT                            # [DIM, CAP]

Every GEMM runs on the PE in fp8 DoubleRow mode (2 k-chunks per instruction,
0.5 cycles/row = 4x the f32r rate) with 3 compensation terms sharing one PSUM
accumulation group:

    W @ X ~= W_hi@X_hi + W_lo@X_hi + W_hi@X_lo        (drop W_lo@X_lo ~ 1e-3)

The cross terms pair (W_lo_k, X_hi_k) and (W_hi_k, X_lo_k) as the two planes
of a single DoubleRow instruction, so each GEMM costs 3 fp8 K-passes = 0.75x
the one-pass f32r time. End-to-end rel err ~1e-3 (vs 2e-2 budget).

The SwiGLU intermediate h is re-quantized to two-level fp8 on-device
(scalar: silu, vector: scaled-mul + residual-sub, gpsimd: hi cast) and stays
SBUF-resident — no DRAM staging between the two phases. Host scatter-adds
gate-weighted outputs back to the full [N_TOKENS, DIM] result.
"""

import os
import numpy as np
import ml_dtypes

import concourse.bacc as bacc
import concourse.mybir as mybir
import concourse.tile as tile
from concourse.bass_utils import run_bass_kernel_spmd

# Problem constants (hardcoded per harness contract)
NUM_EXPERTS = 8
DIM = 2048
FFN = 5632
N_CORES = 8
KD = DIM // 128    # 16 contraction chunks for GEMM1/3; output chunks for GEMM2
KF = FFN // 128    # 44 ffn chunks

F32 = mybir.dt.float32
FP8 = mybir.dt.float8e4
NP_FP8 = ml_dtypes.float8_e4m3
DR = mybir.MatmulPerfMode.DoubleRow

# Power-of-2 quantization scales. fp8 relative precision is scale-free; the
# scale only positions the distribution inside e4m3's normal range
# (2^-6 .. 240). hi and lo share the scale so all matmul terms accumulate in
# one PSUM group.
S_X = 16.0       # x ~ N(0,1): max|x|*16 ~ 90
S_W = 1024.0     # w ~ 0.02*N(0,1): max|w|*1024 ~ 115
S_H = 4.0        # h = silu(g1)*g3, |h| <~ 20: max|h|*4 ~ 80
INV_SXW = 1.0 / (S_X * S_W)          # PSUM -> g dequant (2^-14)
G3_TO_HT = S_H / (S_X * S_W)         # PSUM g3 -> g3*S_H   (2^-12)
INV_SWH = 1.0 / (S_W * S_H)          # PSUM -> y dequant   (2^-12)

# Compiled program cache keyed by CAP
_PROGRAMS = {}

# Filled by the last kernel() call when BASS_KERNEL_TRACE=1 (for test.py)
LAST_EXEC_NS = None


def _split(total, hi):
    """Split `total` into near-equal (offset, size) parts, each <= hi and
    a multiple of 8 (last part takes the remainder). Near-equal keeps every
    part's matmul streaming time above the PE sequencer's per-instruction
    issue cost (~302 cols at DoubleRow rate)."""
    n = -(-total // hi)
    base = (total // n) & ~7
    parts = [base] * (n - 1) + [total - base * (n - 1)]
    out = []
    t0 = 0
    for p in parts:
        out.append((t0, p))
        t0 += p
    return out


def _split_greedy(total, hi, lo=256):
    """Greedy split: hi-sized parts, each in [lo, hi]."""
    assert total >= lo
    parts = []
    rem = total
    while rem > hi:
        take = hi if rem - hi >= lo else rem - lo
        parts.append(take)
        rem -= take
    parts.append(rem)
    out = []
    t0 = 0
    for p in parts:
        out.append((t0, p))
        t0 += p
    return out


def _build_program(cap, tiling="equal", w3_ring="sp", xc_mode="hyb2",
                   wchunk="first3", wbufs=2, warmup=0, w2chunk=4, f0i="x",
                   psum_store=False):
    nc = bacc.Bacc("TRN2", target_bir_lowering=False, debug=False,
                   num_devices=N_CORES)

    xc_d = nc.dram_tensor("xc", [KD, 128, 2, cap], FP8, kind="ExternalInput")
    w1l_d = nc.dram_tensor("w1l", [KF, 128, KD, 2, 128], FP8,
                           kind="ExternalInput")
    w3l_d = nc.dram_tensor("w3l", [KF, 128, KD, 2, 128], FP8,
                           kind="ExternalInput")
    w2l_d = nc.dram_tensor("w2l", [KD, 128, KF, 2, 128], FP8,
                           kind="ExternalInput")
    yt_d = nc.dram_tensor("yt", [KD, 128, cap], F32, kind="ExternalOutput")

    silu = mybir.ActivationFunctionType.Silu
    copyf = mybir.ActivationFunctionType.Copy
    mult = mybir.AluOpType.mult
    sub = mybir.AluOpType.subtract
    p1t = _split(cap, 512) if tiling == "equal" else _split_greedy(cap, 512)

    with tile.TileContext(nc) as tc:
        with (
            tc.tile_pool(name="xc", bufs=1) as xpool,
            tc.tile_pool(name="hc", bufs=1) as hpool,
            tc.tile_pool(name="w13", bufs=wbufs) as wpool,
            tc.tile_pool(name="w2", bufs=2) as w2pool,
            tc.tile_pool(name="hst", bufs=3) as spool,
            tc.tile_pool(name="yo", bufs=3) as ypool,
            tc.tile_pool(name="ps1", bufs=3, space="PSUM") as psum1,
            tc.tile_pool(name="ps2", bufs=2, space="PSUM") as psum2,
        ):
            # x (hi+lo fp8) loads on SWDGE (gpsimd ring) — SP and ACT rings
            # carry the w1/w3 weight streams. hi planes stream first: the
            # hi-hi matmul pass only reads plane 0, so the PE can start while
            # the lo planes are still in flight.
            xc_s = xpool.tile([128, KD, 2, cap], FP8)
            if xc_mode == "plane":
                for j in range(2):
                    for k in range(KD):
                        nc.gpsimd.dma_start(xc_s[:, k, j], xc_d[k, :, j])
            elif xc_mode == "spring":
                # first two k-chunks ahead of the f=0 weights on the SP ring;
                # the rest interleave after them (see f-loop below). One ring
                # means the serial DMA mover serves strictly in need-order.
                for k in range(2):
                    nc.sync.dma_start(xc_s[:, k], xc_d[k])
            elif xc_mode == "hyb3":
                # k0,k1 are the ring's first two descriptors; the rest follow
                # in lockstep-consumption order (see f=0 below)
                for k in range(2):
                    nc.sync.dma_start(xc_s[:, k], xc_d[k])
            elif xc_mode == "hyb5":
                pass  # k0,k1 issued inside the f=0 paired order below
            elif xc_mode == "hyb7":
                # k0 via SWDGE; k1 becomes ring descriptor #2 (f=0 below)
                nc.gpsimd.dma_start(xc_s[:, 0], xc_d[0])
            elif xc_mode in ("hyb", "hyb4", "hyb2"):
                # first two k-chunks via SWDGE (cheap descriptors, parallel
                # to the HWDGE ring) so the first matmul starts ~2us earlier;
                # the bulk still streams in need-order on the SP ring.
                for k in range(2):
                    nc.gpsimd.dma_start(xc_s[:, k], xc_d[k])
            else:
                for k in range(KD):
                    nc.gpsimd.dma_start(xc_s[:, k], xc_d[k])

            if warmup:
                # PE p-state pre-warm: ramp the PE clock on a memset tile
                # while the DMA pipes fill (a PE idle gap resets the clock
                # ramp; entering real work at full clock saves the ~2x
                # mid-pstate tax on the first ~3us of matmuls).
                wup = xpool.tile([128, 2, 512], FP8, name="wup")
                nc.vector.memset(wup[:], 0.0)
                wps = psum1.tile([128, 512], F32, tag="h1p")
                for i in range(warmup):
                    nc.tensor.matmul(wps[:], wup[:, :, :128], wup[:],
                                     start=True, stop=True, perf_mode=DR)
            # SBUF-resident two-level fp8 SwiGLU intermediate
            hc = hpool.tile([128, KF, 2, cap], FP8)

            def dr_group(out_ap, wc, xs, nk, t0, tn, xfirst=False):
                """3-term compensated GEMM: one PSUM accumulation group of
                nk/2 hi-hi DoubleRow matmuls + nk cross DoubleRow matmuls.
                xfirst emits the k0/k1 cross matmuls first (each needs one
                x chunk, not a pair) so the PE can start one chunk earlier
                during the cold-start x stream."""
                if xfirst:
                    for k in range(2):
                        nc.tensor.matmul(
                            out_ap, wc[:, k], xs[:, k, :, t0:t0 + tn],
                            start=(k == 0), stop=False, perf_mode=DR)
                for j in range(nk // 2):
                    nc.tensor.matmul(
                        out_ap, wc[:, 2 * j:2 * j + 2, 1, :],
                        xs[:, 2 * j:2 * j + 2, 0, t0:t0 + tn],
                        start=(j == 0 and not xfirst), stop=False,
                        perf_mode=DR)
                for k in range(2 if xfirst else 0, nk):
                    # plane0: W_lo_k x X_hi_k, plane1: W_hi_k x X_lo_k
                    nc.tensor.matmul(
                        out_ap, wc[:, k], xs[:, k, :, t0:t0 + tn],
                        start=False, stop=(k == nk - 1), perf_mode=DR)

            # ---- Phase 1: hT = silu(w1 @ xT) * (w3 @ xT), fp8 in SBUF ----
            for f in range(KF):
                # w1 on the SP ring, w3 on the ACT ring. The first few tiles
                # are chunked so the PE's first matmuls wait on a quarter-tile
                # transfer; beyond that one descriptor per tile minimizes ring
                # sequencing.
                nch = 4 if (wchunk == "all4" or f < 3) else 1
                w3_eng = nc.scalar if w3_ring == "act" else nc.sync
                w1c = wpool.tile([128, KD, 2, 128], FP8, tag="w1c")
                w3c = wpool.tile([128, KD, 2, 128], FP8, tag="w3c")
                if f == 0 and xc_mode in ("hyb2", "hyb3", "hyb5", "hyb7"):
                    # lockstep-consumption DMA order for the interleaved f=0
                    # emission: [w1c_c, w3c_c, xc ks(c)] so the serial ring
                    # delivers operands exactly as the 6 PSUM groups consume
                    # them (k0,k1 arrive via SWDGE)
                    ksets = [(2, 4), (4, 8), (8, 12), (12, KD)]
                    for c in range(4):
                        sl = slice(c * 4, (c + 1) * 4)
                        nc.sync.dma_start(w1c[:, sl], w1l_d[f][:, sl])
                        if c == 0 and xc_mode == "hyb5":
                            nc.sync.dma_start(xc_s[:, 0], xc_d[0])
                            nc.sync.dma_start(xc_s[:, 1], xc_d[1])
                        elif c == 0 and xc_mode == "hyb7":
                            nc.sync.dma_start(xc_s[:, 1], xc_d[1])
                        nc.sync.dma_start(w3c[:, sl], w3l_d[f][:, sl])
                        for k in range(*ksets[c]):
                            nc.sync.dma_start(xc_s[:, k], xc_d[k])
                else:
                    for c in range(nch):
                        sl = slice(c * KD // nch, (c + 1) * KD // nch)
                        nc.sync.dma_start(w1c[:, sl], w1l_d[f][:, sl])
                    for c in range(nch):
                        sl = slice(c * KD // nch, (c + 1) * KD // nch)
                        w3_eng.dma_start(w3c[:, sl], w3l_d[f][:, sl])
                if f == 0 and xc_mode in ("spring", "hyb"):
                    for k in range(2, KD):
                        nc.sync.dma_start(xc_s[:, k], xc_d[k])
                elif f == 0 and xc_mode == "hyb4":
                    # bulk xc as 4 grouped descriptors: fewer ring slots
                    for g in range(4):
                        k0, k1 = 2 + g * 4, min(2 + (g + 1) * 4, KD)
                        nc.sync.dma_start(xc_s[:, k0:k1], xc_d[k0:k1])

                def epilogue(t0, tn, h1p, h3p):
                    s1 = spool.tile([128, 512], F32, tag="s1", name="s1")
                    nc.scalar.activation(s1[:, :tn], h1p[:, :tn], silu,
                                         scale=INV_SXW)
                    ht = spool.tile([128, 512], F32, tag="ht", name="ht")
                    nc.vector.scalar_tensor_tensor(
                        ht[:, :tn], h3p[:, :tn], G3_TO_HT, s1[:, :tn],
                        mult, mult)
                    nc.gpsimd.tensor_copy(hc[:, f, 0, t0:t0 + tn], ht[:, :tn])
                    nc.vector.tensor_tensor(
                        hc[:, f, 1, t0:t0 + tn], ht[:, :tn],
                        hc[:, f, 0, t0:t0 + tn], sub)

                xfirst = (f == 0)
                if f0i == "x" and f == 0:
                    # k-lockstep across all 6 PSUM groups with hi-hi pairs
                    # interleaved after each odd k: consumption (~12 instrs
                    # per 2 chunks) pipelines against the serial DMA mover's
                    # per-chunk delivery, so the PE never stalls on the
                    # cold-start x stream.
                    groups = []
                    for (t0, tn) in p1t:
                        h1p = psum1.tile([128, 512], F32, tag="h1p",
                                         name="h1p")
                        h3p = psum1.tile([128, 512], F32, tag="h3p",
                                         name="h3p")
                        groups.append((h1p, w1c, t0, tn))
                        groups.append((h3p, w3c, t0, tn))
                    for k in range(KD):
                        for (pt, wc, t0, tn) in groups:
                            nc.tensor.matmul(
                                pt[:, :tn], wc[:, k],
                                xc_s[:, k, :, t0:t0 + tn],
                                start=(k == 0), stop=False, perf_mode=DR)
                        if k % 2 == 1:
                            j = k // 2
                            for (pt, wc, t0, tn) in groups:
                                nc.tensor.matmul(
                                    pt[:, :tn], wc[:, 2 * j:2 * j + 2, 1, :],
                                    xc_s[:, 2 * j:2 * j + 2, 0, t0:t0 + tn],
                                    start=False, stop=(k == KD - 1),
                                    perf_mode=DR)
                    for i, (t0, tn) in enumerate(p1t):
                        epilogue(t0, tn, groups[2 * i][0], groups[2 * i + 1][0])
                elif f0i and f == 0:
                    # f=0 runs while xc is still streaming in k order: emit
                    # all 6 PSUM groups (w1/w3 x 3 token tiles) in k-lockstep
                    # so the PE always has work on already-resident k-chunks.
                    groups = []
                    for (t0, tn) in p1t:
                        h1p = psum1.tile([128, 512], F32, tag="h1p",
                                         name="h1p")
                        h3p = psum1.tile([128, 512], F32, tag="h3p",
                                         name="h3p")
                        groups.append((h1p, w1c, t0, tn))
                        groups.append((h3p, w3c, t0, tn))
                    for j in range(KD // 2):
                        for (pt, wc, t0, tn) in groups:
                            nc.tensor.matmul(
                                pt[:, :tn], wc[:, 2 * j:2 * j + 2, 1, :],
                                xc_s[:, 2 * j:2 * j + 2, 0, t0:t0 + tn],
                                start=(j == 0), stop=False, perf_mode=DR)
                    for k in range(KD):
                        for (pt, wc, t0, tn) in groups:
                            nc.tensor.matmul(
                                pt[:, :tn], wc[:, k],
                                xc_s[:, k, :, t0:t0 + tn],
                                start=False, stop=(k == KD - 1), perf_mode=DR)
                    for i, (t0, tn) in enumerate(p1t):
                        epilogue(t0, tn, groups[2 * i][0], groups[2 * i + 1][0])
                else:
                    for (t0, tn) in p1t:
                        h1p = psum1.tile([128, 512], F32, tag="h1p",
                                         name="h1p")
                        dr_group(h1p[:, :tn], w1c, xc_s, KD, t0, tn,
                                 xfirst=xfirst)
                        h3p = psum1.tile([128, 512], F32, tag="h3p",
                                         name="h3p")
                        dr_group(h3p[:, :tn], w3c, xc_s, KD, t0, tn,
                                 xfirst=xfirst)
                        epilogue(t0, tn, h1p, h3p)

            # ---- Phase 2: yT = w2 @ hT ----
            for m in range(KD):
                w2c = w2pool.tile([128, KF, 2, 128], FP8, tag="w2c")
                # chunk-split so the first matmuls of this m start after
                # a fraction of the weight load instead of all of it
                cw = KF // w2chunk
                for c in range(w2chunk):
                    nc.sync.dma_start(w2c[:, c * cw:(c + 1) * cw],
                                      w2l_d[m][:, c * cw:(c + 1) * cw])
                for ti, (t0, tn) in enumerate(p1t):
                    yp = psum2.tile([128, 512], F32, tag="yp")
                    dr_group(yp[:, :tn], w2c, hc, KF, t0, tn)
                    yo = ypool.tile([128, 512], F32, tag="yo")
                    nc.scalar.activation(yo[:, :tn], yp[:, :tn], copyf,
                                         scale=INV_SWH)
                    nc.scalar.dma_start(yt_d[m][:, t0:t0 + tn],
                                        yo[:, :tn])

    nc.compile()
    return nc


def _quant2(a, scale):
    """Two-level e4m3 quantization at a shared power-of-2 scale.
    Returns (hi, lo) fp8 arrays with hi + lo ~= a * scale."""
    s = (a * scale).astype(np.float32)
    hi = s.astype(NP_FP8)
    lo = (s - hi.astype(np.float32)).astype(NP_FP8)
    return hi, lo


def _pack_w(w, scale):
    """[R, C] f32 -> [R//128, 128, C//128, 2, 128] fp8 with
    out[r, p, k, j, m] = Wq_j[r*128+m, k*128+p]; j=0 lo, j=1 hi."""
    R, C = w.shape
    hi, lo = _quant2(w, scale)
    q = np.stack([lo, hi])                       # [2, R, C]
    q = q.reshape(2, R // 128, 128, C // 128, 128)
    return np.ascontiguousarray(q.transpose(1, 4, 3, 0, 2))


def kernel(x, expert_indices, expert_weights, w1, w2, w3):
    global LAST_EXEC_NS
    x = np.ascontiguousarray(np.asarray(x, dtype=np.float32))
    routing = np.asarray(expert_indices)
    probs = np.asarray(expert_weights, dtype=np.float32)
    w1 = np.asarray(w1, dtype=np.float32)
    w2 = np.asarray(w2, dtype=np.float32)
    w3 = np.asarray(w3, dtype=np.float32)
    n_tokens = x.shape[0]

    idxs = [np.flatnonzero(routing[:, e]) for e in range(NUM_EXPERTS)]
    max_count = max(len(i) for i in idxs)
    # capacity = max expert load, rounded to 8 (device requires alignment;
    # r4 hit NRT_EXEC_UNIT_UNRECOVERABLE; PE time scales linearly with cap)
    cap = max(512, -(-max_count // 8) * 8)
    assert cap <= 2304, f"unexpectedly imbalanced routing: max_count={max_count}"

    if cap not in _PROGRAMS:
        _PROGRAMS[cap] = _build_program(cap)
    nc = _PROGRAMS[cap]

    def _prep(e):
        idx = idxs[e]
        xt = np.zeros((DIM, cap), dtype=np.float32)
        xt[:, :len(idx)] = x[idx].T
        xhi, xlo = _quant2(xt, S_X)
        xq = np.stack([xhi, xlo])                # [2, DIM, cap]
        xq = xq.reshape(2, KD, 128, cap)
        return {
            "xc": np.ascontiguousarray(xq.transpose(1, 2, 0, 3)),
            "w1l": _pack_w(w1[e], S_W),
            "w3l": _pack_w(w3[e], S_W),
            "w2l": _pack_w(w2[e], S_W),
        }

    from concurrent.futures import ThreadPoolExecutor
    with ThreadPoolExecutor(max_workers=NUM_EXPERTS) as pool:
        in_maps = list(pool.map(_prep, range(NUM_EXPERTS)))

    trace = os.environ.get("BASS_KERNEL_TRACE", "0") == "1"
    if trace:
        import importlib.util
        if importlib.util.find_spec("antenv") is None or importlib.util.find_spec(
                "antenv.axon_hooks") is None:
            trace = False  # NTFF hook unavailable in this environment
    res = run_bass_kernel_spmd(
        nc, in_maps, core_ids=list(range(N_CORES)),
        trace=trace, trace_cores=list(range(N_CORES)) if trace else None,
    )
    LAST_EXEC_NS = res.exec_time_ns

    out = np.zeros((n_tokens, DIM), dtype=np.float32)
    for e in range(NUM_EXPERTS):
        idx = idxs[e]
        y_t = res.results[e]["yt"].reshape(DIM, cap)[:, :len(idx)]
        out[idx] += probs[idx, e][:, None] * y_t.T
    return out
